# revision 1
# baseline (speedup 1.0000x reference)
"""Deformable transformer decoder layer for Trainium2 (8 NeuronCores).

Sharding: data-parallel over batch B=4 x token-half (2) -> 8 cores.
All dense projections (QKV/out projs, conv taps, FFN, value proj) run on
device through one reusable Bass tiled-matmul program (M=1024, K=256,
N=512, fp32, PSUM-accumulated over K); softmax/layernorm/bilinear-gather
glue runs on host between device invocations.
"""

import sys

import numpy as np

for _p in ("/opt/trn_rl_repo",):
    if _p not in sys.path:
        sys.path.insert(0, _p)

import concourse.bass as bass
import concourse.mybir as mybir
from concourse.bass_utils import run_bass_kernel_spmd
from concourse.tile import TileContext

D = 256
H = 8
DH = D // H
L = 4
P = 4
NADJ = 4
DFF = 1024
SPATIAL_SHAPES = [(100, 134), (50, 67), (25, 34), (13, 17)]
LEVEL_START = [0, 13400, 16750, 17600]
LV = 17821
B, NQ, NP = 4, 100, 20
T = NQ * NP  # 2000 tokens per batch

MT, KT, NT = 1024, 256, 512  # device matmul tile: out[MT,NT] = xt.T @ w
NCORES = 8

_NC = None
_EXEC_NS = 0  # accumulated device busy-time estimate (ns), see _dev_mm
_NCALLS = 0


def _get_nc():
    global _NC
    if _NC is not None:
        return _NC
    f32 = mybir.dt.float32
    nc = bass.Bass()
    xw = nc.declare_dram_parameter("xw", [KT, MT + NT], f32, isOutput=False)
    out = nc.declare_dram_parameter("out", [MT, NT], f32, isOutput=True)
    KB = KT // 128
    MB = MT // 128
    import contextlib

    stack = contextlib.ExitStack()
    xwt = stack.enter_context(nc.sbuf_tensor("xwt", [128, KB, MT + NT], f32))
    obig = stack.enter_context(nc.sbuf_tensor("obig", [128, MB, NT], f32))
    psums = [
        stack.enter_context(nc.psum_tensor(f"ps{i}", [128, NT], f32))
        for i in range(8)
    ]
    dsem = stack.enter_context(nc.semaphore("dsem"))
    pes = stack.enter_context(nc.semaphore("pes"))
    psem = stack.enter_context(nc.semaphore("psem"))
    with stack, nc.Block() as block:

        @block.sync
        def _(sync):
            sync.dma_start(
                out=xwt[:], in_=xw[:].rearrange("(a p) m -> p a m", p=128)
            ).then_inc(dsem, 16)
            sync.wait_ge(psem, MB)
            sync.dma_start(
                out=out[:].rearrange("(a p) m -> p a m", p=128), in_=obig[:]
            ).then_inc(dsem, 16)
            sync.wait_ge(dsem, 32)

        @block.tensor
        def _(tensor):
            tensor.wait_ge(dsem, 16)
            for mb in range(MB):
                for kb in range(KB):
                    inst = tensor.matmul(
                        psums[mb][:],
                        lhsT=xwt[:, kb, mb * 128 : (mb + 1) * 128],
                        rhs=xwt[:, kb, MT : MT + NT],
                        start=(kb == 0),
                        stop=(kb == KB - 1),
                    )
                inst.then_inc(pes, 1)

        @block.scalar
        def _(scalar):
            for mb in range(MB):
                scalar.wait_ge(pes, mb + 1)
                scalar.copy(obig[:, mb, :], psums[mb][:]).then_inc(psem, 1)
    _NC = nc
    return nc


_DEV_OK = True
_FAST_OK = True


def _dev_mm(jobs):
    """jobs: list (<=8) of (X [m<=1024, 256], W [256, n<=512]) fp32.
    Runs on the NeuronCores; falls back to host numpy if the device
    path is unavailable in the calling environment."""
    global _DEV_OK
    if not _DEV_OK:
        return [np.asarray(X, np.float32) @ np.asarray(W, np.float32) for X, W in jobs]
    try:
        return _dev_mm_hw(jobs)
    except Exception as e:  # device unavailable -> host fallback
        sys.stderr.write(f"device path failed ({type(e).__name__}: {e}); "
                         "falling back to host matmul\n")
        _DEV_OK = False
        return _dev_mm(jobs)


_RUNNER = None


def _get_runner():
    """Build the sharded PJRT executable once and reuse it for every
    invocation (run_bass_kernel_spmd re-traces/jits per call, ~1s each)."""
    global _RUNNER
    if _RUNNER is not None:
        return _RUNNER
    import jax
    from jax.experimental.shard_map import shard_map
    from jax.sharding import Mesh, PartitionSpec

    import concourse.bass2jax as b2j

    b2j.install_neuronx_cc_hook()
    nc = _get_nc()
    out_aval = jax.core.ShapedArray((MT, NT), np.float32)
    # derive parameter order from BIR allocations exactly like
    # run_bass_via_pjrt does (the neuronx_cc_hook checks it)
    pname = nc.partition_id_tensor.name if nc.partition_id_tensor else None
    in_names, out_names = [], []
    for alloc in nc.m.functions[0].allocations:
        if not isinstance(alloc, mybir.MemoryLocationSet):
            continue
        name = alloc.memorylocations[0].name
        if alloc.kind == "ExternalInput":
            if name != pname:
                in_names.append(name)
        elif alloc.kind == "ExternalOutput":
            out_names.append(name)
    in_names = tuple(in_names + out_names + ([pname] if pname else []))

    def _body(xw_in, out_zero):
        operands = [xw_in, out_zero]
        if pname:
            operands.append(b2j.partition_id_tensor())
        outs = b2j._bass_exec_p.bind(
            *operands,
            out_avals=(out_aval,),
            in_names=in_names,
            out_names=tuple(out_names),
            lowering_input_output_aliases=(),
            sim_require_finite=True,
            sim_require_nnan=True,
            nc=nc,
        )
        return tuple(outs)

    devices = jax.devices()[:NCORES]
    mesh = Mesh(np.asarray(devices), ("core",))
    _RUNNER = jax.jit(
        shard_map(
            _body,
            mesh=mesh,
            in_specs=(PartitionSpec("core"),) * 2,
            out_specs=(PartitionSpec("core"),),
            check_rep=False,
        ),
        donate_argnums=(1,),
        keep_unused=True,
    )
    return _RUNNER


def _dev_mm_fast(in_maps):
    runner = _get_runner()
    xw_cat = np.concatenate([m["xw"] for m in in_maps], axis=0)
    zeros = np.zeros((NCORES * MT, NT), np.float32)
    out = np.asarray(runner(xw_cat, zeros)[0]).reshape(NCORES, MT, NT)
    return [{"out": out[c]} for c in range(NCORES)]


def _dev_mm_hw(jobs):
    global _EXEC_NS, _NCALLS
    nc = _get_nc()
    in_maps = []
    shapes = []
    for c in range(NCORES):
        if c < len(jobs):
            X, W = jobs[c]
        else:
            X = np.zeros((1, KT), np.float32)
            W = np.zeros((KT, 1), np.float32)
        m, n = X.shape[0], W.shape[1]
        shapes.append((m, n))
        xwp = np.zeros((KT, MT + NT), np.float32)
        xwp[:, :m] = X.T
        xwp[:, MT : MT + n] = W
        in_maps.append({"xw": xwp})
    global _FAST_OK
    try:
        if not _FAST_OK:
            raise RuntimeError("fast path disabled")
        results = _dev_mm_fast(in_maps)
    except Exception as e:
        if _FAST_OK:
            sys.stderr.write(
                f"fast runner failed ({type(e).__name__}: {e}); using spmd path\n"
            )
            _FAST_OK = False
        res = run_bass_kernel_spmd(nc, in_maps, list(range(NCORES)))
        if res.exec_time_ns:
            _EXEC_NS += int(res.exec_time_ns)
        results = res.results
    _NCALLS += 1
    outs = []
    for c in range(len(jobs)):
        m, n = shapes[c]
        outs.append(np.asarray(results[c]["out"])[:m, :n])
    return outs


def _mm_tokens(X, W):
    """X [B, T, 256] @ W [256, n<=512] -> [B, T, n]; 8 cores = B x half."""
    n = W.shape[1]
    half = T // 2
    jobs = []
    for c in range(NCORES):
        b, g = divmod(c, 2)
        jobs.append((X[b, g * half : (g + 1) * half], W))
    outs = _dev_mm(jobs)
    res = np.empty((B, T, n), np.float32)
    for c in range(NCORES):
        b, g = divmod(c, 2)
        res[b, g * half : (g + 1) * half] = outs[c]
    return res


def _mm_rows(X, W):
    """X [R, 256] @ W [256, n<=512] -> [R, n], chunked over 8 cores."""
    R = X.shape[0]
    n = W.shape[1]
    chunks = [(s, min(s + MT, R)) for s in range(0, R, MT)]
    res = np.empty((R, n), np.float32)
    for r0 in range(0, len(chunks), NCORES):
        grp = chunks[r0 : r0 + NCORES]
        outs = _dev_mm([(X[a:b2], W) for a, b2 in grp])
        for (a, b2), o in zip(grp, outs):
            res[a:b2] = o
    return res


def _layer_norm(x, g, b, eps=1e-5):
    m = x.mean(-1, keepdims=True)
    v = ((x - m) ** 2).mean(-1, keepdims=True)
    return ((x - m) / np.sqrt(v + eps) * g + b).astype(np.float32)


def _softmax(x, axis=-1):
    m = x.max(axis=axis, keepdims=True)
    e = np.exp(x - m)
    return (e / e.sum(axis=axis, keepdims=True)).astype(np.float32)


def _attention(qp, kp, vp):
    """qp/kp/vp: [G, S, 256] projected q/k/v; returns [G, S, 256]."""
    G, S, _ = qp.shape
    sp = lambda t: t.reshape(G, S, H, DH).transpose(0, 2, 1, 3)
    q, k, v = sp(qp), sp(kp), sp(vp)
    att = _softmax(np.einsum("ghqd,ghkd->ghqk", q, k) / np.sqrt(DH), -1)
    o = np.einsum("ghqk,ghkd->ghqd", att, v)
    return o.transpose(0, 2, 1, 3).reshape(G, S, D).astype(np.float32)


def _bilinear(vflat, Hl, Wl, x, y):
    x0 = np.floor(x)
    y0 = np.floor(y)
    lx = x - x0
    ly = y - y0
    x0 = x0.astype(np.int64)
    y0 = y0.astype(np.int64)
    out = 0.0
    for dy, wy in ((0, 1.0 - ly), (1, ly)):
        for dx, wx in ((0, 1.0 - lx), (1, lx)):
            xi = x0 + dx
            yi = y0 + dy
            valid = (xi >= 0) & (xi < Wl) & (yi >= 0) & (yi < Hl)
            idx = np.clip(yi, 0, Hl - 1) * Wl + np.clip(xi, 0, Wl - 1)
            gs = np.take_along_axis(vflat, idx[..., None], axis=1)
            out = out + gs * (wx * wy * valid)[..., None]
    return out.astype(np.float32)


def kernel(
    tgt, query_pos, query_pos_anchor, reference_points, src,
    src_spatial_shapes, level_start_index,
    ia_wi, ia_bi, ia_wo, ia_bo,
    cc_w, cc_b, bn_g, bn_b, bn_m, bn_v,
    ni_g, ni_b, mf_w, mf_b, nf_g, nf_b,
    in_wi, in_bi, in_wo, in_bo, nin_g, nin_b,
    so_w, so_b, aw_w, aw_b, vp_w, vp_b, op_w, op_b, nc_g, nc_b,
    l1_w, l1_b, l2_w, l2_b, n3_g, n3_b,
):
    f = lambda a: np.asarray(a, np.float32)
    tgt = f(tgt)
    qp = f(query_pos)
    qpa = f(query_pos_anchor)
    ref = f(reference_points)
    src = f(src)

    x0 = tgt.reshape(B, T, D)
    qpf = qp.reshape(B, T, D)
    qpaf = qpa.reshape(B, T, D)

    # ---- intra attention (sequences = NP points within each (b, nq)) ----
    q_in = x0 + qpf
    qk = _mm_tokens(q_in, f(ia_wi)[: 2 * D].T)  # [B,T,512] -> q|k
    vproj = _mm_tokens(x0, f(ia_wi)[2 * D :].T)
    qprj = qk[..., :D] + f(ia_bi)[:D]
    kprj = qk[..., D:] + f(ia_bi)[D : 2 * D]
    vprj = vproj + f(ia_bi)[2 * D :]
    o = _attention(
        qprj.reshape(B * NQ, NP, D),
        kprj.reshape(B * NQ, NP, D),
        vprj.reshape(B * NQ, NP, D),
    ).reshape(B, T, D)
    t_att = _mm_tokens(o, f(ia_wo).T) + f(ia_bo)

    # ---- circular conv over NP + BN + ReLU ----
    sc = (x0 + qpf).reshape(B, NQ, NP, D)
    xp = np.concatenate([sc[:, :, -NADJ:], sc, sc[:, :, :NADJ]], axis=2)
    conv = np.zeros((B, T, D), np.float32)
    ccw = f(cc_w)
    for t in range(2 * NADJ + 1):
        Xt = xp[:, :, t : t + NP, :].reshape(B, T, D)
        conv += _mm_tokens(Xt, ccw[:, :, t].T)
    conv = conv + f(cc_b)
    conv = (conv - f(bn_m)) / np.sqrt(f(bn_v) + 1e-5) * f(bn_g) + f(bn_b)
    t_cc = np.maximum(conv, 0.0)

    y = x0 + _layer_norm(t_att + t_cc, f(ni_g), f(ni_b))
    mf = _mm_tokens(y, f(mf_w).T) + f(mf_b)
    y = y + _layer_norm(mf, f(nf_g), f(nf_b))

    # ---- inter attention (sequences = NQ instances for each (b, np)) ----
    q_in2 = y + qpaf
    qk2 = _mm_tokens(q_in2, f(in_wi)[: 2 * D].T)
    vproj2 = _mm_tokens(y, f(in_wi)[2 * D :].T)
    qprj2 = (qk2[..., :D] + f(in_bi)[:D]).reshape(B, NQ, NP, D)
    kprj2 = (qk2[..., D:] + f(in_bi)[D : 2 * D]).reshape(B, NQ, NP, D)
    vprj2 = (vproj2 + f(in_bi)[2 * D :]).reshape(B, NQ, NP, D)
    # group by np: [B*NP, NQ, D]
    tonp = lambda a: a.transpose(0, 2, 1, 3).reshape(B * NP, NQ, D)
    o2 = _attention(tonp(qprj2), tonp(kprj2), tonp(vprj2))
    o2 = o2.reshape(B, NP, NQ, D).transpose(0, 2, 1, 3).reshape(B, T, D)
    t2 = _mm_tokens(o2, f(in_wo).T) + f(in_bo)
    ti = _layer_norm(y + t2, f(nin_g), f(nin_b))

    # ---- deformable cross attention ----
    qc = ti + qpf
    proj = _mm_tokens(qc, np.concatenate([f(so_w), f(aw_w)], 0).T)  # [B,T,384]
    offsets = (proj[..., : H * L * P * 2] + f(so_b)).reshape(B, T, H, L, P, 2)
    aw = _softmax(
        (proj[..., H * L * P * 2 :] + f(aw_b)).reshape(B, T, H, L * P), -1
    ).reshape(B, T, H, L, P)
    value = (_mm_rows(src.reshape(B * LV, D), f(vp_w).T) + f(vp_b)).reshape(
        B, LV, H, DH
    )
    refq = ref.reshape(B, T, L, 2)
    normalizer = np.array(
        [[wl, hl] for hl, wl in SPATIAL_SHAPES], np.float32
    )  # [L,2] = (W,H)
    loc = (
        refq[:, :, None, :, None, :]
        + offsets / normalizer[None, None, None, :, None, :]
    )
    out_s = np.zeros((B, T, H, DH), np.float32)
    for lvl, (Hl, Wl) in enumerate(SPATIAL_SHAPES):
        s = LEVEL_START[lvl]
        vflat = (
            value[:, s : s + Hl * Wl]
            .transpose(0, 2, 1, 3)
            .reshape(B * H, Hl * Wl, DH)
        )
        g = 2.0 * loc[:, :, :, lvl] - 1.0
        x = ((g[..., 0] + 1.0) / 2.0) * Wl - 0.5
        y_ = ((g[..., 1] + 1.0) / 2.0) * Hl - 0.5
        x = x.transpose(0, 2, 1, 3).reshape(B * H, T * P)
        y_ = y_.transpose(0, 2, 1, 3).reshape(B * H, T * P)
        samp = _bilinear(vflat, Hl, Wl, x, y_).reshape(B, H, T, P, DH)
        wgt = aw[:, :, :, lvl].transpose(0, 2, 1, 3)  # [B,H,T,P]
        out_s += np.einsum("nhqp,nhqpd->nqhd", wgt, samp).astype(np.float32)
    sampled = out_s.reshape(B, T, D)
    t2d = _mm_tokens(sampled, f(op_w).T) + f(op_b)
    tgt2 = _layer_norm(ti + t2d, f(nc_g), f(nc_b))

    # ---- FFN ----
    h1 = np.concatenate(
        [
            _mm_tokens(tgt2, f(l1_w)[:512].T),
            _mm_tokens(tgt2, f(l1_w)[512:].T),
        ],
        axis=-1,
    ) + f(l1_b)
    h1 = np.maximum(h1, 0.0)
    h2 = np.zeros((B, T, D), np.float32)
    l2 = f(l2_w)
    for kb in range(DFF // D):
        h2 += _mm_tokens(
            np.ascontiguousarray(h1[..., kb * D : (kb + 1) * D]),
            l2[:, kb * D : (kb + 1) * D].T,
        )
    h2 = h2 + f(l2_b)
    out = _layer_norm(tgt2 + h2, f(n3_g), f(n3_b))
    return out.reshape(B, NQ, NP, D).astype(np.float32)

